# revision 4
# baseline (speedup 1.0000x reference)
"""ColBERT intra-batch MaxSim scoring kernel for 8 Trainium2 NeuronCores.

Math (see reference):
  Q = l2norm(q_hidden @ W.T)                       [B, LQ, DIM]
  D = l2norm(d_hidden @ W.T); D masked             [B, LD, DIM]
  sim[b,c,q,k] = Q[b,q]·D[c,k]; masked k -> -inf
  out[b,c] = sum_q max_k sim

Sharding: docs (dim c) are sharded 16-per-core; q_hidden/W replicated.
Each core computes its [B, 16] slice of the score matrix.

Device-side trickery:
  * Host pre-transposes activations to [HID, tokens] so every matmul has
    its contraction dim on partitions (no on-device transposes).
  * The doc mask is folded away on the host: each doc's valid tokens are
    gathered to the front and the tail is padded with copies of the doc's
    first valid token. Duplicated valid columns never change a max, so the
    device kernel needs no masking at all, and the sim matmul + max only
    cover NV <= LD columns per doc (NV = max valid count, rounded to 32).
  * Q is NOT normalized before the sim matmul: max_k is invariant under a
    positive per-query scale, so 1/|Q| is applied to the per-doc maxes.
  * D norms: ones-matmul gives sumsq as a [1, NVT] row; sqrt (ACT) +
    2-ULP approximate reciprocal (DVE) give 1/|D| which is broadcast to
    128 partitions with a K=1 ones outer-product matmul and multiplied in.
  * Projections and the big sim matmul run in float32r (full PE rate);
    norm/sum matmuls stay float32 for accuracy.
"""

import os

import numpy as np

B, LQ, LD, HID, DIM = 128, 32, 256, 768, 128
NCORES = 8
DPC = B // NCORES          # docs per core
TQ = B * LQ                # total query tokens
KC = HID // 128            # contraction chunks for the projection

# dtype of the projection / sim matmuls: "f32r" (fast) or "f32" (exact)
PROJ_MODE = os.environ.get("KERNEL_PROJ_MODE", "f32r")
SIM_MODE = os.environ.get("KERNEL_SIM_MODE", "f32r")


def _build_program(NV):
    import concourse.bass as bass
    import concourse.tile as tile
    from concourse import bacc, mybir

    f32 = mybir.dt.float32
    f32r = mybir.dt.float32r
    AF = mybir.ActivationFunctionType
    AX = mybir.AxisListType
    ALU = mybir.AluOpType

    NVT = DPC * NV          # compacted doc tokens per core
    NVH = NVT // 2          # half (8 docs) — one PSUM sim tile
    assert NVT % 512 == 0
    NQCH = TQ // 512        # q-projection column chunks
    NDCH = NVT // 512       # d-projection column chunks
    NTT = TQ // 128         # sim lhsT tiles (query-token tiles)
    BPT = 128 // LQ         # batch entries per query-token tile

    def rdt(mode):
        return f32r if mode == "f32r" else f32

    proj_dt = rdt(PROJ_MODE)
    sim_dt = rdt(SIM_MODE)

    nc = bacc.Bacc(
        "TRN2",
        target_bir_lowering=False,
        debug=False,
        num_devices=NCORES,
    )

    qT_d = nc.dram_tensor("qT", [HID, TQ], proj_dt, kind="ExternalInput")
    dT_d = nc.dram_tensor("dT", [HID, NVT], proj_dt, kind="ExternalInput")
    wT_d = nc.dram_tensor("wT", [HID, DIM], proj_dt, kind="ExternalInput")
    qso_d = nc.dram_tensor("qso", [128, BPT], f32, kind="ExternalInput")
    onescol_d = nc.dram_tensor("onescol", [128, 1], f32, kind="ExternalInput")
    onesrow_d = nc.dram_tensor("onesrow", [1, 128], f32, kind="ExternalInput")
    out_d = nc.dram_tensor("out", [B, DPC], f32, kind="ExternalOutput")

    with tile.TileContext(nc) as tc, tc.tile_pool(name="persist", bufs=1) as persist:
        # --- constants + big persistent SBUF tensors -----------------------
        wt = persist.tile([128, KC, DIM], proj_dt, name="wt")
        qso = persist.tile([128, BPT], f32, name="qso")
        onescol = persist.tile([128, 1], f32, name="onescol")
        onesrow = persist.tile([1, 128], f32, name="onesrow")
        QT = persist.tile([128, TQ], sim_dt, name="QT")        # q-proj, [d, t], unnormalized
        DTraw = persist.tile([128, NVT], f32, name="DTraw")  # d-proj, [d, t]
        DTn = persist.tile([128, NVT], sim_dt, name="DTn")      # normalized d-proj
        invnQ = persist.tile([128, NTT], f32, name="invnQ")  # 1/|Q| per query token
        normQ = persist.tile([128, NTT], f32, name="normQ")
        ssqD_row = persist.tile([1, NVT], f32, name="ssqD_row")
        invnD_row = persist.tile([1, NVT], f32, name="invnD_row")
        rowtmp = persist.tile([1, NVT], f32, name="rowtmp")
        outstage = persist.tile([BPT, NTT * DPC], f32, name="outstage")

        for k in range(KC):
            nc.sync.dma_start(wt[:, k, :], wT_d[k * 128:(k + 1) * 128, :])
        nc.sync.dma_start(qso[:], qso_d[:, :])
        nc.sync.dma_start(onescol[:], onescol_d[:, :])
        nc.sync.dma_start(onesrow[:], onesrow_d[:, :])

        # ---------------- phase D: project doc tokens ----------------------
        with (
            tc.tile_pool(name="dt_pool", bufs=1) as dt_pool,
            tc.tile_pool(name="psD", bufs=2, space="PSUM") as psD,
            tc.tile_pool(name="ssD", bufs=2, space="PSUM") as ssD,
            tc.tile_pool(name="sq_pool", bufs=2) as sqD_pool,
        ):
            dt = dt_pool.tile([128, KC, NVT], proj_dt, name="dt")
            for k in range(KC):
                nc.sync.dma_start(dt[:, k, :], dT_d[k * 128:(k + 1) * 128, :])

            for j in range(NDCH):
                sl = slice(j * 512, (j + 1) * 512)
                psd = psD.tile([128, 512], f32, name="psd", tag="psd")
                for k in range(KC):
                    nc.tensor.matmul(
                        psd[:],
                        wt[:, k, :],
                        dt[:, k, sl],
                        start=(k == 0),
                        stop=(k == KC - 1),
                    )
                nc.scalar.copy(DTraw[:, sl], psd[:])
                sq = sqD_pool.tile([128, 512], f32, name="sqd", tag="sq")
                nc.scalar.activation(sq[:], psd[:], AF.Square)
                ssd = ssD.tile([1, 512], f32, name="ssd", tag="ssd")
                nc.tensor.matmul(ssd[:], onescol[:], sq[:], start=True, stop=True)
                nc.scalar.copy(ssqD_row[:, sl], ssd[:])

        # 1/|D| row: sqrt then ~2ULP reciprocal (single-lane, overlapped
        # with the Q phase below).
        nc.scalar.activation(rowtmp[:], ssqD_row[:], AF.Sqrt)
        nc.vector.reciprocal_approx_accurate(invnD_row[:], rowtmp[:], ssqD_row[:])

        # ---------------- phase Q: project query tokens --------------------
        with (
            tc.tile_pool(name="qt_pool", bufs=1) as qt_pool,
            tc.tile_pool(name="psQ", bufs=2, space="PSUM") as psQ,
            tc.tile_pool(name="ssQ", bufs=1, space="PSUM") as ssQ,
            tc.tile_pool(name="sqQ_pool", bufs=2) as sqQ_pool,
            tc.tile_pool(name="psB", bufs=2, space="PSUM") as psB,
        ):
            qt = qt_pool.tile([128, KC, TQ], proj_dt, name="qt")
            for k in range(KC):
                nc.sync.dma_start(qt[:, k, :], qT_d[k * 128:(k + 1) * 128, :])

            ssq = ssQ.tile([128, NTT], f32, name="ssq")
            for j in range(NQCH):
                sl = slice(j * 512, (j + 1) * 512)
                psq = psQ.tile([128, 512], f32, name="psq", tag="psq")
                for k in range(KC):
                    nc.tensor.matmul(
                        psq[:],
                        wt[:, k, :],
                        qt[:, k, sl],
                        start=(k == 0),
                        stop=(k == KC - 1),
                    )
                nc.scalar.copy(QT[:, sl], psq[:])
                sq = sqQ_pool.tile([128, 512], f32, name="sqq", tag="sqq")
                nc.scalar.activation(sq[:], psq[:], AF.Square)
                for s in range(4):
                    col = j * 4 + s
                    nc.tensor.matmul(
                        ssq[:, col:col + 1],
                        sq[:, s * 128:(s + 1) * 128],
                        onescol[:],
                        start=True,
                        stop=True,
                    )

            # 1/|Q| per query token, [t-tile partition layout]
            nc.scalar.activation(normQ[:], ssq[:], AF.Sqrt)
            nc.vector.reciprocal(invnQ[:], normQ[:])

            # broadcast 1/|D| across partitions and scale D
            for j in range(NDCH):
                sl = slice(j * 512, (j + 1) * 512)
                psb = psB.tile([128, 512], f32, name="psb", tag="psb")
                nc.tensor.matmul(
                    psb[:], onesrow[:], invnD_row[:, sl], start=True, stop=True
                )
                nc.vector.tensor_tensor(DTn[:, sl], DTraw[:, sl], psb[:], op=ALU.mult)

        # ---------------- phase S: sim matmul + masked max ------------------
        nsim = []
        off = 0
        while off < NVH:
            step = min(512, NVH - off)
            nsim.append((off, step))
            off += step

        with (
            tc.tile_pool(name="psS", bufs=2, space="PSUM") as psS,
            tc.tile_pool(name="psO", bufs=1, space="PSUM") as psO,
            tc.tile_pool(name="m_pool", bufs=2) as m_pool,
        ):
            psout = psO.tile([BPT, NTT * DPC], f32, name="psout")
            for t in range(NTT):
                lq = QT[:, t * 128:(t + 1) * 128]
                mall = m_pool.tile([128, DPC], f32, name="mall", tag="mall")
                for h in range(2):
                    base = h * NVH
                    ps = psS.tile([128, NVH], f32, name="pssim", tag="pssim")
                    for (off, step) in nsim:
                        nc.tensor.matmul(
                            ps[:, off:off + step],
                            lq,
                            DTn[:, base + off:base + off + step],
                            start=True,
                            stop=True,
                        )
                    nc.vector.reduce_max(
                        mall[:, h * (DPC // 2):(h + 1) * (DPC // 2)],
                        ps[:].rearrange("p (g v) -> p g v", v=NV),
                        axis=AX.X,
                    )
                nc.vector.tensor_scalar_mul(mall[:], mall[:], invnQ[:, t:t + 1])
                nc.tensor.matmul(
                    psout[:, t * DPC:(t + 1) * DPC],
                    qso[:],
                    mall[:],
                    start=True,
                    stop=True,
                )
            nc.scalar.copy(outstage[:], psout[:])
            nc.sync.dma_start(
                out_d[:, :].rearrange("(t f) c -> f t c", f=BPT),
                outstage[:].rearrange("f (t c) -> f t c", c=DPC),
            )

    nc.compile()
    return nc


def _host_prep(q_hidden, d_hidden, W, d_mask):
    q = np.ascontiguousarray(np.asarray(q_hidden, dtype=np.float32))
    d = np.ascontiguousarray(np.asarray(d_hidden, dtype=np.float32))
    w = np.ascontiguousarray(np.asarray(W, dtype=np.float32))
    mask = np.asarray(d_mask, dtype=bool)

    nv = mask.sum(axis=1)
    NV = int(-(-max(int(nv.max()), 1) // 32) * 32)
    NV = min(NV, ((LD + 31) // 32) * 32)

    # per-doc gather indices: valid tokens first, padded with the first
    # valid token (duplicates never change a max)
    idx = np.zeros((B, NV), dtype=np.intp)
    for c in range(B):
        v = np.flatnonzero(mask[c])
        row = np.full(NV, v[0], dtype=np.intp)
        row[:min(len(v), NV)] = v[:NV]
        idx[c] = row

    dG = d[np.arange(B)[:, None], idx, :]          # [B, NV, HID]

    qT = np.ascontiguousarray(q.reshape(TQ, HID).T)     # [HID, TQ]
    wT = np.ascontiguousarray(w.T)                      # [HID, DIM]
    dT_cores = []
    for m in range(NCORES):
        blk = dG[m * DPC:(m + 1) * DPC].reshape(DPC * NV, HID)
        dT_cores.append(np.ascontiguousarray(blk.T))    # [HID, DPC*NV]

    qso = np.zeros((128, 128 // LQ), dtype=np.float32)
    for p in range(128):
        qso[p, p // LQ] = 1.0
    onescol = np.ones((128, 1), dtype=np.float32)
    onesrow = np.ones((1, 128), dtype=np.float32)
    return NV, qT, wT, dT_cores, qso, onescol, onesrow


def kernel(q_hidden, d_hidden, W, d_mask):
    from concourse.bass_utils import run_bass_kernel_spmd

    NV, qT, wT, dT_cores, qso, onescol, onesrow = _host_prep(
        q_hidden, d_hidden, W, d_mask
    )
    nc = _build_program(NV)

    in_maps = [
        {
            "qT": qT,
            "dT": dT_cores[m],
            "wT": wT,
            "qso": qso,
            "onescol": onescol,
            "onesrow": onesrow,
        }
        for m in range(NCORES)
    ]
    res = run_bass_kernel_spmd(nc, in_maps, core_ids=list(range(NCORES)))
    out = np.concatenate(
        [res.results[m]["out"] for m in range(NCORES)], axis=1
    )
    return np.ascontiguousarray(out.astype(np.float32))


# revision 9
# speedup vs baseline: 1.2841x; 1.2841x over previous
"""ColBERT intra-batch MaxSim scoring kernel for 8 Trainium2 NeuronCores.

Math (see reference):
  Q = l2norm(q_hidden @ W.T)                       [B, LQ, DIM]
  D = l2norm(d_hidden @ W.T); D masked             [B, LD, DIM]
  sim[b,c,q,k] = Q[b,q]·D[c,k]; masked k -> -inf
  out[b,c] = sum_q max_k sim

Sharding: docs (dim c) are sharded 16-per-core; q_hidden/W replicated.
Each core computes its [B, 16] slice of the score matrix.

Device-side structure:
  * Host pre-transposes activations to [HID, tokens] so every matmul has
    its contraction dim on partitions (no on-device transposes).
  * The doc mask is folded away on the host: each doc's valid tokens are
    gathered to the front and the tail is padded with copies of the doc's
    first valid token. Duplicated valid columns never change a max, so the
    device kernel needs no masking at all, and the sim matmul + max only
    cover NV <= LD columns per doc (NV = max valid count, rounded to 8).
  * Q is NOT normalized before the sim matmul: max_k is invariant under a
    positive per-query scale, so 1/|Q| is folded into the block-ones
    lhsT of the final query-sum matmul.
  * D norms: ones-matmul gives sumsq as a [1, NVT] row; sqrt (ACT) +
    2-ULP approximate reciprocal (DVE) give 1/|D| which is broadcast to
    128 partitions with a K=1 ones outer-product matmul and multiplied in.
  * Projections run in float32r (full PE rate); the sim matmul and the
    sum-of-squares matmuls run in bf16 (inputs bf16, fp32 PSUM accum);
    norm values / maxes stay fp32 end to end.
"""

import os

import numpy as np

B, LQ, LD, HID, DIM = 128, 32, 256, 768, 128
NCORES = 8
DPC = B // NCORES          # docs per core
TQ = B * LQ                # total query tokens
KC = HID // 128            # contraction chunks for the projection

# dtypes of the projection / sim matmuls (see _dt_of)
PROJ_MODE = os.environ.get("KERNEL_PROJ_MODE", "f32r")
SIM_MODE = os.environ.get("KERNEL_SIM_MODE", "bf16")


def _chunks(total, step):
    """[(off, len)] cut at `step` boundaries — a matmul's PSUM output must
    stay inside a single 512-float bank, so chunks may never straddle one."""
    return [(o, min(step, total - o)) for o in range(0, total, step)]


def _build_program(NV):
    import concourse.bass as bass  # noqa: F401
    import concourse.tile as tile
    from concourse import bacc, mybir

    f32 = mybir.dt.float32
    AF = mybir.ActivationFunctionType
    AX = mybir.AxisListType
    ALU = mybir.AluOpType

    def _dt_of(mode):
        return {
            "f32": f32,
            "f32r": mybir.dt.float32r,
            "bf16": mybir.dt.bfloat16,
        }[mode]

    proj_dt = _dt_of(PROJ_MODE)
    sim_dt = _dt_of(SIM_MODE)
    # the sum-of-squares matmuls: bf16 when the sim path is bf16, else plain
    # fp32 (fp32r has ISA restrictions that reject tiny-N matmuls)
    sq_dt = sim_dt if SIM_MODE == "bf16" else f32

    NVT = DPC * NV          # compacted doc tokens per core
    NVH = NVT // 2          # half (8 docs) — one PSUM sim tile
    NQCH = TQ // 512        # q-projection column chunks
    NTT = TQ // 128         # sim lhsT tiles (query-token tiles)
    BPT = 128 // LQ         # batch entries per query-token tile
    QG = 1024               # qt DMA column-group width
    NQG = TQ // QG
    d_chunks = _chunks(NVT, 512)   # d-projection column chunks
    s_chunks = _chunks(NVH, 512)   # sim matmul N chunks per half

    nc = bacc.Bacc(
        "TRN2",
        target_bir_lowering=False,
        debug=False,
        num_devices=NCORES,
    )

    qT_d = nc.dram_tensor("qT", [HID, TQ], proj_dt, kind="ExternalInput")
    dT_d = nc.dram_tensor("dT", [HID, NVT], proj_dt, kind="ExternalInput")
    wT_d = nc.dram_tensor("wT", [HID, DIM], proj_dt, kind="ExternalInput")
    qso_d = nc.dram_tensor("qso", [128, BPT], f32, kind="ExternalInput")
    onescol_d = nc.dram_tensor("onescol", [128, 1], sq_dt, kind="ExternalInput")
    onesrow_d = nc.dram_tensor("onesrow", [1, 128], f32, kind="ExternalInput")
    out_d = nc.dram_tensor("out", [B, DPC], f32, kind="ExternalOutput")
    dbg = os.environ.get("KERNEL_DEBUG_OUT", "0") == "1"
    if dbg:
        dbgQT_d = nc.dram_tensor("dbg_QT", [128, TQ], sim_dt, kind="ExternalOutput")
        dbgDTraw_d = nc.dram_tensor("dbg_DTraw", [128, NVT], f32, kind="ExternalOutput")
        dbgDTn_d = nc.dram_tensor("dbg_DTn", [128, NVT], sim_dt, kind="ExternalOutput")
        dbginvnQ_d = nc.dram_tensor("dbg_invnQ", [128, NTT], f32, kind="ExternalOutput")
        dbglhsQ_d = nc.dram_tensor("dbg_lhsQ", [128, NTT, BPT], f32, kind="ExternalOutput")

    with tile.TileContext(nc) as tc, tc.tile_pool(name="persist", bufs=1) as per:
        # --- constants + persistent SBUF tensors ---------------------------
        wt = per.tile([128, KC, DIM], proj_dt, name="wt")
        qso = per.tile([128, BPT], f32, name="qso")
        onescol = per.tile([128, 1], sq_dt, name="onescol")
        onesrow = per.tile([1, 128], f32, name="onesrow")
        QT = per.tile([128, TQ], sim_dt, name="QT")       # q-proj [d, t] unnormalized
        DTraw = per.tile([128, NVT], f32, name="DTraw")   # d-proj [d, t]
        DTn = per.tile([128, NVT], sim_dt, name="DTn")    # normalized d-proj
        invnQ = per.tile([128, NTT], f32, name="invnQ")   # 1/|Q| per query token
        normQ = per.tile([128, NTT], f32, name="normQ")
        lhsQ = per.tile([128, NTT, BPT], f32, name="lhsQ")  # blockones * 1/|Q|
        ssqD_row = per.tile([1, NVT], f32, name="ssqD_row")
        invnD_row = per.tile([1, NVT], f32, name="invnD_row")
        rowtmp = per.tile([1, NVT], f32, name="rowtmp")
        outstage = per.tile([BPT, NTT * DPC], f32, name="outstage")

        for k in range(KC):
            nc.sync.dma_start(wt[:, k, :], wT_d[k * 128:(k + 1) * 128, :])
        nc.sync.dma_start(qso[:], qso_d[:, :])
        nc.sync.dma_start(onescol[:], onescol_d[:, :])
        nc.sync.dma_start(onesrow[:], onesrow_d[:, :])

        # ---------------- phase D: project doc tokens ----------------------
        # k-outer accumulation into one wide PSUM tensor so compute starts
        # as soon as the first dT k-chunk lands.
        with (
            tc.tile_pool(name="dt_pool", bufs=1) as dt_pool,
            tc.tile_pool(name="psD", bufs=1, space="PSUM") as psD,
            tc.tile_pool(name="ssD", bufs=2, space="PSUM") as ssD,
            tc.tile_pool(name="sqD_pool", bufs=2) as sqD_pool,
        ):
            dts = []
            for k in range(KC):
                dtk = dt_pool.tile([128, NVT], proj_dt, name=f"dt{k}", tag=f"dt{k}")
                nc.sync.dma_start(dtk[:], dT_d[k * 128:(k + 1) * 128, :])
                dts.append(dtk)

            psd = psD.tile([128, NVT], f32, name="psd")
            for k in range(KC):
                for (off, ln) in d_chunks:
                    nc.tensor.matmul(
                        psd[:, off:off + ln],
                        wt[:, k, :],
                        dts[k][:, off:off + ln],
                        start=(k == 0),
                        stop=(k == KC - 1),
                    )
            for (off, ln) in d_chunks:
                sl = slice(off, off + ln)
                nc.scalar.copy(DTraw[:, sl], psd[:, sl])
                sq = sqD_pool.tile([128, 512], sq_dt, name="sqd", tag="sq")
                nc.scalar.activation(sq[:, :ln], psd[:, sl], AF.Square)
                ssd = ssD.tile([1, 512], f32, name="ssd", tag="ssd")
                nc.tensor.matmul(
                    ssd[:, :ln], onescol[:], sq[:, :ln], start=True, stop=True
                )
                nc.scalar.copy(ssqD_row[:, sl], ssd[:, :ln])

        # 1/|D| row: sqrt then ~2ULP reciprocal (single-lane, overlapped
        # with the Q phase below).
        nc.scalar.activation(rowtmp[:], ssqD_row[:], AF.Sqrt)
        nc.vector.reciprocal_approx_accurate(invnD_row[:], rowtmp[:], ssqD_row[:])

        # ---------------- phase Q: project query tokens --------------------
        with (
            tc.tile_pool(name="qt_pool", bufs=1) as qt_pool,
            tc.tile_pool(name="psQ", bufs=2, space="PSUM") as psQ,
            tc.tile_pool(name="ssQ", bufs=1, space="PSUM") as ssQ,
            tc.tile_pool(name="sqQ_pool", bufs=2) as sqQ_pool,
            tc.tile_pool(name="psB", bufs=2, space="PSUM") as psB,
        ):
            # qt DMAs land jg-major so the first 1024 columns of all six
            # k-chunks arrive first and Q-proj can start early.
            qts = {}
            for jg in range(NQG):
                for k in range(KC):
                    t_ = qt_pool.tile(
                        [128, QG], proj_dt, name=f"qt{k}_{jg}", tag=f"qt{k}_{jg}"
                    )
                    eng = nc.sync if (k % 2 == 0) else nc.scalar
                    eng.dma_start(t_[:], qT_d[k * 128:(k + 1) * 128,
                                              jg * QG:(jg + 1) * QG])
                    qts[(k, jg)] = t_

            ssq = ssQ.tile([128, NTT], f32, name="ssq")
            for j in range(NQCH):
                sl = slice(j * 512, (j + 1) * 512)
                jg, r = divmod(j * 512, QG)
                psq = psQ.tile([128, 512], f32, name="psq", tag="psq")
                for k in range(KC):
                    nc.tensor.matmul(
                        psq[:],
                        wt[:, k, :],
                        qts[(k, jg)][:, r:r + 512],
                        start=(k == 0),
                        stop=(k == KC - 1),
                    )
                nc.scalar.copy(QT[:, sl], psq[:])
                sq = sqQ_pool.tile([128, 512], sq_dt, name="sqq", tag="sqq")
                nc.scalar.activation(sq[:], psq[:], AF.Square)
                for s in range(4):
                    col = j * 4 + s
                    nc.tensor.matmul(
                        ssq[:, col:col + 1],
                        sq[:, s * 128:(s + 1) * 128],
                        onescol[:],
                        start=True,
                        stop=True,
                    )
                # per-chunk 1/|Q| + weighted block-ones lhsT so the sim phase
                # can start before the whole Q projection finishes
                csl = slice(j * 4, (j + 1) * 4)
                nc.scalar.activation(normQ[:, csl], ssq[:, csl], AF.Sqrt)
                nc.vector.reciprocal(invnQ[:, csl], normQ[:, csl])
                nc.vector.tensor_tensor(
                    lhsQ[:, csl, :],
                    qso[:].unsqueeze(1).broadcast_to((128, 4, BPT)),
                    invnQ[:, csl].unsqueeze(2).broadcast_to((128, 4, BPT)),
                    op=ALU.mult,
                )

            # broadcast 1/|D| across partitions and scale D
            for (off, ln) in d_chunks:
                sl = slice(off, off + ln)
                psb = psB.tile([128, 512], f32, name="psb", tag="psb")
                nc.tensor.matmul(
                    psb[:, :ln], onesrow[:], invnD_row[:, sl], start=True, stop=True
                )
                nc.vector.tensor_tensor(
                    DTn[:, sl], DTraw[:, sl], psb[:, :ln], op=ALU.mult
                )

        # ---------------- phase S: sim matmul + masked max ------------------
        with (
            tc.tile_pool(name="psS", bufs=2, space="PSUM") as psS,
            tc.tile_pool(name="psO", bufs=1, space="PSUM") as psO,
            tc.tile_pool(name="m_pool", bufs=2) as m_pool,
        ):
            psout = psO.tile([BPT, NTT * DPC], f32, name="psout")
            for t in range(NTT):
                lq = QT[:, t * 128:(t + 1) * 128]
                mall = m_pool.tile([128, DPC], f32, name="mall", tag="mall")
                for h in range(2):
                    base = h * NVH
                    ps = psS.tile([128, NVH], f32, name="pssim", tag="pssim")
                    for (off, ln) in s_chunks:
                        nc.tensor.matmul(
                            ps[:, off:off + ln],
                            lq,
                            DTn[:, base + off:base + off + ln],
                            start=True,
                            stop=True,
                        )
                    nc.vector.reduce_max(
                        mall[:, h * (DPC // 2):(h + 1) * (DPC // 2)],
                        ps[:].rearrange("p (g v) -> p g v", v=NV),
                        axis=AX.X,
                    )
                nc.tensor.matmul(
                    psout[:, t * DPC:(t + 1) * DPC],
                    lhsQ[:, t, :],
                    mall[:],
                    start=True,
                    stop=True,
                )
            if dbg:
                nc.sync.dma_start(dbgQT_d[:, :], QT[:])
                nc.sync.dma_start(dbgDTraw_d[:, :], DTraw[:])
                nc.sync.dma_start(dbgDTn_d[:, :], DTn[:])
                nc.sync.dma_start(dbginvnQ_d[:, :], invnQ[:])
                nc.sync.dma_start(dbglhsQ_d[:, :, :], lhsQ[:])
            nc.scalar.copy(outstage[:], psout[:])
            nc.sync.dma_start(
                out_d[:, :].rearrange("(t f) c -> f t c", f=BPT),
                outstage[:].rearrange("f (t c) -> f t c", c=DPC),
            )

    nc.compile()
    return nc


def _host_prep(q_hidden, d_hidden, W, d_mask):
    import ml_dtypes

    q = np.ascontiguousarray(np.asarray(q_hidden, dtype=np.float32))
    d = np.ascontiguousarray(np.asarray(d_hidden, dtype=np.float32))
    w = np.ascontiguousarray(np.asarray(W, dtype=np.float32))
    mask = np.asarray(d_mask, dtype=bool)

    nv = mask.sum(axis=1)
    NV = int(-(-max(int(nv.max()), 16) // 8) * 8)
    NV = min(NV, ((LD + 7) // 8) * 8)

    # per-doc gather indices: valid tokens first, padded with the first
    # valid token (duplicates never change a max)
    idx = np.zeros((B, NV), dtype=np.intp)
    for c in range(B):
        v = np.flatnonzero(mask[c])
        row = np.full(NV, v[0], dtype=np.intp)
        row[:min(len(v), NV)] = v[:NV]
        idx[c] = row

    dG = d[np.arange(B)[:, None], idx, :]          # [B, NV, HID]

    qT = np.ascontiguousarray(q.reshape(TQ, HID).T)     # [HID, TQ]
    wT = np.ascontiguousarray(w.T)                      # [HID, DIM]
    dT_cores = []
    for m in range(NCORES):
        blk = dG[m * DPC:(m + 1) * DPC].reshape(DPC * NV, HID)
        dT_cores.append(np.ascontiguousarray(blk.T))    # [HID, DPC*NV]

    qso = np.zeros((128, 128 // LQ), dtype=np.float32)
    for p in range(128):
        qso[p, p // LQ] = 1.0
    ones_dt = ml_dtypes.bfloat16 if SIM_MODE == "bf16" else np.float32
    onescol = np.ones((128, 1), dtype=ones_dt)
    onesrow = np.ones((1, 128), dtype=np.float32)
    return NV, qT, wT, dT_cores, qso, onescol, onesrow


def kernel(q_hidden, d_hidden, W, d_mask):
    from concourse.bass_utils import run_bass_kernel_spmd

    NV, qT, wT, dT_cores, qso, onescol, onesrow = _host_prep(
        q_hidden, d_hidden, W, d_mask
    )
    nc = _build_program(NV)

    in_maps = [
        {
            "qT": qT,
            "dT": dT_cores[m],
            "wT": wT,
            "qso": qso,
            "onescol": onescol,
            "onesrow": onesrow,
        }
        for m in range(NCORES)
    ]
    res = run_bass_kernel_spmd(nc, in_maps, core_ids=list(range(NCORES)))
    out = np.concatenate(
        [res.results[m]["out"] for m in range(NCORES)], axis=1
    )
    return np.ascontiguousarray(out.astype(np.float32))
